# revision 54
# baseline (speedup 1.0000x reference)
"""Multi-head attention (B=2, S=2048, D=1024, H=16) on 8 Trainium2 NeuronCores.

Sharding: batch x head-group. Core c handles batch b = c//4 and heads
[4*(c%4), 4*(c%4)+4) (a 256-wide slice of the QKV projection output and the
matching 256-row slice of Wo). Each core computes its partial output
projection; a 4-way ReduceScatter per batch group sums the partials and
leaves each core with a [512, 1024] row block of the final output, which the
host reassembles.

Per-core dataflow (all matmul operands fp16, fp32 PSUM accumulation):
  - x^T arrives pre-transposed from the host (plain contiguous DMA), split
    across both HWDGE queues (SP: K/V path, ACT: Q path) so the K
    projection starts ~3us in.
  - Q^T, K^T feature-major; K^T zero-padded per head so scores run as
    full-128-contraction plain matmuls. All matmuls stay in plain mode -
    no PE tiling-mode switches anywhere.
  - Scores computed transposed (S^T[k, q] = K_h @ Q_h^T); softmax without
    max-subtraction (exp on ScalarE, 1/sqrt(dh) scale folded in).
  - attn@V via ones-augmented V so each accumulation also produces the
    softmax denominator in a spare PSUM row. A per-pair indicator matmul
    broadcasts the two denominator rows to all 128 partitions; fast DVE
    reciprocal + two muls produce normalized attn^T.
  - The whole kernel is software-pipelined around the ScalarE exp stream
    (the throughput floor): each head-pair's phase-1 score/exp loop has the
    previous pair's attn@V, the denominator matmul, the V/Q projections and
    the previous chunk's output projection injected into its PE slack.
  - Per-q-chunk ReduceScatter overlaps the next chunk's attention.
"""

import numpy as np

import concourse.bass as bass  # noqa: F401  (engine namespaces via nc)
import concourse.mybir as mybir
import concourse.tile as tile
from concourse import bacc
from concourse.bass import _add_dep_helper
from concourse.bass_utils import run_bass_kernel_spmd

F32 = mybir.dt.float32
F16 = mybir.dt.float16
F8 = mybir.dt.float8e4
NP_F8 = mybir.dt.np(F8)
AF = mybir.ActivationFunctionType

B, S, D = 2, 2048, 1024
H, DH = 16, 64
NCORES = 8
GPB = 4                # cores per batch group
HPC = H // GPB         # heads per core
DS = HPC * DH          # 256: per-core slice of the projection output
P = 128
NDT = D // P           # 8 d_model tiles
NTT = S // P           # 16 token tiles
QCH = 512              # q-chunk (PSUM bank = 512 fp32)
NQC = S // QCH         # 4
NKT = S // P           # 16 k tiles
NKP = NKT // 2         # 8 phase-1 iterations per pair
SCALE = float(1.0 / np.sqrt(DH))

REPLICA_GROUPS = [[0, 1, 2, 3], [4, 5, 6, 7]]

_CACHED_NC = None


def _build_module():
    nc = bacc.Bacc("TRN2", target_bir_lowering=False, debug=False,
                   num_devices=NCORES)

    # host-prearranged [partition, dmodel-tile * token] so the load DMA is a
    # straight copy (one contiguous 32KB block per partition)
    xq_d = nc.dram_tensor("xq", [P, NDT * S], F16, kind="ExternalInput")
    xk_d = nc.dram_tensor("xk", [P, NDT * S], F16, kind="ExternalInput")
    xv_d = nc.dram_tensor("xv", [P, NDT * S], F16, kind="ExternalInput")
    wq_d = nc.dram_tensor("wq", [D, DS], F16, kind="ExternalInput")
    wk_d = nc.dram_tensor("wk", [D, DS], F16, kind="ExternalInput")
    wv_d = nc.dram_tensor("wv", [D, DS], F16, kind="ExternalInput")
    wo_d = nc.dram_tensor("wo", [DS, D], F16, kind="ExternalInput")
    bq_d = nc.dram_tensor("bq", [DS, 1], F32, kind="ExternalInput")
    bk_d = nc.dram_tensor("bk", [DS, 1], F32, kind="ExternalInput")
    bv_d = nc.dram_tensor("bv", [1, DS], F32, kind="ExternalInput")
    bo_d = nc.dram_tensor("bo", [1, D], F32, kind="ExternalInput")

    out_d = nc.dram_tensor("out", [S // GPB, D], F16, kind="ExternalOutput")
    partial_cs = [nc.dram_tensor(f"partial{j}", [4 * P, D], F16)
                  for j in range(4)]
    rs_cs = [nc.dram_tensor(f"rs_out{j}", [P, D], F16)
             for j in range(4)]

    with tile.TileContext(nc) as tc:
        with (
            tc.tile_pool(name="cst", bufs=1) as cst,
            tc.tile_pool(name="xt", bufs=1) as xtp,
            tc.tile_pool(name="exp", bufs=22) as expp,
            tc.tile_pool(name="rcp", bufs=2) as rcpp,
            tc.tile_pool(name="osb", bufs=3) as osbp,
            tc.tile_pool(name="psB", bufs=3, space="PSUM") as psB,
            tc.tile_pool(name="psC", bufs=1, space="PSUM") as psC,
        ):
            # Total PE ordering: chain every matmul to its predecessor
            # (nosync = scheduling-order only). All matmuls are plain mode.
            _real_matmul = nc.tensor.matmul
            _prev_mm = {"inst": None}

            def mm(out, lhsT, rhs, **kw):
                inst = _real_matmul(out, lhsT, rhs, **kw)
                if _prev_mm["inst"] is not None:
                    _add_dep_helper(
                        inst.ins, _prev_mm["inst"].ins,
                        sync=False, reason="pe-order")
                _prev_mm["inst"] = inst
                return inst

            # ---- constants + inputs: SP queue = K/V path, ACT queue = Q --
            wq_t = cst.tile([P, NDT, DS], F16, tag="wq")
            wk_t = cst.tile([P, NDT, DS], F16, tag="wk")
            wv_t = cst.tile([P, NDT, DS], F16, tag="wv")
            wo_t = cst.tile([P, 2, D], F16, tag="wo")
            bq_t = cst.tile([P, 2, 1], F32, tag="bq")
            bk_t = cst.tile([P, 2, 1], F32, tag="bk")
            bv_row = cst.tile([1, DS], F32, tag="bvr")
            bo_row = cst.tile([1, D], F32, tag="bor")

            def load_xt(x_d, eng, tag):
                # one big DMA per tensor: per-DMA overhead dominates over
                # bytes, so 8 tile-DMAs are ~2x slower than one rearrange.
                # xv reuses xk's buffer (tag) once the K projection drains.
                big = xtp.tile([P, NDT, S], F16, tag=tag, name=f"x{tag}")
                # two half-DMAs: the projection's first accumulation pass
                # (dt 0-3) can start while the second half is in flight
                h = NDT // 2
                eng.dma_start(big[:, 0:h, :], x_d[:, 0:h * S])
                eng.dma_start(big[:, h:NDT, :], x_d[:, h * S:])
                return [big[:, dt, :] for dt in range(NDT)]

            # Single queue, strictly in order of first use (HBM bandwidth is
            # shared across queues, so splitting only delays the K/Q path).
            nc.sync.dma_start(wk_t[:], wk_d.rearrange("(a p) n -> p a n", p=P))
            xt_k = load_xt(xk_d, nc.sync, "bigA")
            nc.sync.dma_start(wq_t[:], wq_d.rearrange("(a p) n -> p a n", p=P))
            xt_q = load_xt(xq_d, nc.sync, "bigB")
            nc.sync.dma_start(wv_t[:], wv_d.rearrange("(a p) n -> p a n", p=P))
            nc.sync.dma_start(wo_t[:], wo_d.rearrange("(a p) n -> p a n", p=P))
            nc.sync.dma_start(bo_row[:], bo_d[:])
            xt_v = load_xt(xv_d, nc.sync, "bigA")

            nc.scalar.dma_start(
                bq_t[:], bq_d.rearrange("(a p) o -> p a o", p=P))
            nc.scalar.dma_start(
                bk_t[:], bk_d.rearrange("(a p) o -> p a o", p=P))
            nc.scalar.dma_start(bv_row[:], bv_d[:])

            bv_b = cst.tile([P, 2, 2, DH], F32, tag="bvb")
            bo_b = cst.tile([P, D], F32, tag="bob")
            nc.gpsimd.partition_broadcast(bv_b[:], bv_row[:])
            nc.gpsimd.partition_broadcast(bo_b[:], bo_row[:])

            # ---- resident activations ----
            qt_t = cst.tile([P, 2, S], F16, tag="qt")   # Q^T  (pair, t)
            kz_t = cst.tile([P, HPC, S], F16, tag="kz")  # zero-padded K^T
            nc.vector.memset(kz_t[:], 0.0)
            an_t = cst.tile([P, 2, S], F16, tag="an")   # attn_norm^T

            # Ones-augmented V, token-major (see module docstring).
            vaA_t = cst.tile([P, NTT, 2, P], F16, tag="vaA")
            vaB_t = cst.tile([P, NTT, 2, P], F16, tag="vaB")
            nc.vector.memset(vaA_t[:], 0.0)
            nc.vector.memset(vaB_t[:], 0.0)
            nc.vector.memset(vaA_t[:, :, :, DH:DH + 1], 1.0)
            nc.vector.memset(vaB_t[:, :, :, 0:1], 1.0)

            # Indicator for the denominator select+broadcast matmul.
            ind_t = cst.tile([P, P], F16, tag="ind")
            nc.vector.memset(ind_t[:], 0.0)
            nc.vector.memset(ind_t[DH:DH + 1, 0:DH], 1.0)
            nc.vector.memset(ind_t[0:1, DH:P], 1.0)

            # Denominator staging rows (rows 0/64 per pair; rest must stay
            # zero - the den-matmul contracts all 128 partitions and fp16
            # garbage can be NaN).
            dsb_t = cst.tile([P, 2, QCH], F16, tag="dsb")
            nc.vector.memset(dsb_t[:], 0.0)

            # ---- building blocks ----
            def qk_proj_chunk(dst, w_t, b_t, xt, tci):
                """One 512-token chunk of the feature-major Q/K projection."""
                ts0 = tci * QCH
                ps = psB.tile([P, 2 * QCH], F32, tag="sc")
                for dot in range(2):
                    col = slice(dot * QCH, (dot + 1) * QCH)
                    for dt in range(NDT):
                        mm(
                            ps[:, col],
                            w_t[:, dt, dot * P:(dot + 1) * P],
                            xt[dt][:, ts0:ts0 + QCH],
                            start=(dt == 0), stop=(dt == NDT - 1),
                        )
                if dst is qt_t:
                    for dot in range(2):
                        nc.vector.tensor_scalar_add(
                            dst[:, dot, ts0:ts0 + QCH],
                            ps[:, dot * QCH:(dot + 1) * QCH],
                            b_t[:, dot, :])
                else:  # kz_t: per-head 64-row slices, rest stays zero
                    for h in range(HPC):
                        rows = slice((h % 2) * 64, (h % 2) * 64 + 64)
                        dot = h // 2
                        nc.vector.tensor_scalar_add(
                            kz_t[rows, h, ts0:ts0 + QCH],
                            ps[rows, dot * QCH:(dot + 1) * QCH],
                            b_t[rows, dot, :])

            def v_proj_tt(tt):
                """Ones-augmented V projection for one token tile."""
                ps = psB.tile([P, 2, 2, DH], F32, tag="sc")
                for dt in range(NDT):
                    mm(
                        ps[:],
                        xt_v[dt][:, tt * P:(tt + 1) * P],
                        wv_t[:, dt, :],
                        start=(dt == 0), stop=(dt == NDT - 1),
                    )
                nc.vector.tensor_add(
                    vaA_t[:, tt, :, 0:DH], ps[:, :, 0, :], bv_b[:, :, 0, :])
                nc.vector.tensor_add(
                    vaB_t[:, tt, :, DH:2 * DH], ps[:, :, 1, :],
                    bv_b[:, :, 1, :])

            # per-pair state: etiles, pA, pB
            st = {}

            def ph1_kp(qc, pr, kp):
                """Two score matmul pairs + two exps for (qc, pr), k-pair kp."""
                qs = qc * QCH
                h0, h1 = 2 * pr, 2 * pr + 1
                sc0 = psB.tile([P, 2 * QCH], F32, tag="sc")
                sc1 = psB.tile([P, 2 * QCH], F32, tag="sc")
                for hsel, sc in ((h0, sc0), (h1, sc1)):
                    for j in range(2):
                        ks = (2 * kp + j) * P
                        col = slice(j * QCH, (j + 1) * QCH)
                        mm(sc[:, col], kz_t[:, hsel, ks:ks + P],
                           qt_t[:, pr, qs:qs + QCH], start=True, stop=True)
                e0 = expp.tile([P, 2 * QCH], F16, tag="exp")
                e1 = expp.tile([P, 2 * QCH], F16, tag="exp")
                nc.scalar.activation(e0[:], sc0[:], AF.Exp, scale=SCALE)
                nc.scalar.activation(e1[:], sc1[:], AF.Exp, scale=SCALE)
                st[(qc, pr)]["etiles"].append((e0, e1))

            def ph2_kts(qc, pr, kts):
                """attn@V accumulation matmuls for the given k-tiles."""
                s = st[(qc, pr)]
                if s["pA"] is None:
                    s["pA"] = psC.tile([P, QCH], F32, tag="pA", name="pA")
                    s["pB"] = psC.tile([P, QCH], F32, tag="pB", name="pB")
                pA, pB = s["pA"], s["pB"]
                for kt in kts:
                    e0, e1 = s["etiles"][kt // 2]
                    col = slice((kt % 2) * QCH, (kt % 2 + 1) * QCH)
                    stt = (kt == 0)
                    sp = (kt == NKT - 1)
                    mm(pA[0:DH + 1, :], vaA_t[:, kt, pr, 0:DH + 1],
                       e0[:, col], start=stt, stop=sp, skip_group_check=True)
                    mm(pB[:, :], vaB_t[:, kt, pr, :],
                       e1[:, col], start=stt, stop=sp, skip_group_check=True)
                if kts[-1] == NKT - 1:
                    # stage denominator rows for the den-matmul (fp16 SBUF)
                    nc.vector.tensor_copy(
                        dsb_t[DH:DH + 1, pr, :], pA[DH:DH + 1, :])
                    nc.vector.tensor_copy(
                        dsb_t[0:1, pr, :], pB[0:1, :])

            def den_mm(qc, pr):
                """Denominator broadcast matmul + reciprocal."""
                s = st[(qc, pr)]
                dps = psB.tile([P, 2 * QCH], F32, tag="sc")
                mm(dps[:, 0:QCH], ind_t[:], dsb_t[:, pr, :],
                   start=True, stop=True)
                rc = rcpp.tile([P, QCH], F32, tag="rcp")
                nc.vector.reciprocal_approx_fast(rc[:], dps[:, 0:QCH])
                s["rc"] = rc

            def norm_mul(qc, pr):
                s = st[(qc, pr)]
                qs = qc * QCH
                rc, pA, pB = s["rc"], s["pA"], s["pB"]
                nc.vector.tensor_mul(
                    an_t[0:DH, pr, qs:qs + QCH], pA[0:DH, :], rc[0:DH, :])
                nc.vector.tensor_mul(
                    an_t[DH:P, pr, qs:qs + QCH], pB[DH:P, :], rc[DH:P, :])

            def outproj_tt(qc, tt4):
                tt = qc * (QCH // P) + tt4
                po = psB.tile([P, 2 * QCH], F32, tag="sc")
                for half in range(2):
                    for pr in range(2):
                        mm(po[:, half * QCH:(half + 1) * QCH],
                           an_t[:, pr, tt * P:(tt + 1) * P],
                           wo_t[:, pr, half * QCH:(half + 1) * QCH],
                           start=(pr == 0), stop=(pr == 1))
                ob = osbp.tile([P, D], F16, tag="osb")
                nc.vector.tensor_add(ob[:], po[:], bo_b[:])
                nc.sync.dma_start(
                    partial_cs[qc][tt4 * P:(tt4 + 1) * P, :], ob[:])

            def reduce_scatter(qc):
                nc.gpsimd.collective_compute(
                    "ReduceScatter",
                    mybir.AluOpType.add,
                    replica_groups=REPLICA_GROUPS,
                    ins=[partial_cs[qc][:]],
                    outs=[rs_cs[qc][:]],
                )
                nc.sync.dma_start(out_d[qc * P:(qc + 1) * P, :],
                                  rs_cs[qc][:])

            # ---- head: K projection, then Q chunk 0 ----
            for tci in range(NQC):
                qk_proj_chunk(kz_t, wk_t, bk_t, xt_k, tci)
            qk_proj_chunk(qt_t, wq_t, bq_t, xt_q, 0)

            # ---- software-pipelined attention ----
            # pair p's phase 1 (exp-paced) carries, per kp:
            #   - previous pair's attn@V (front-loaded, 4 k-tiles on kp 0-3)
            #   - previous pair's den-matmul at kp 4, norm muls at kp 5
            #   - pair (0,0): V projection, 2 token tiles per kp
            #   - pairs (qc,1): Q projection chunk qc+1, spread over kps
            #   - pairs (qc,0), qc>=1: output projection of chunk qc-1 on
            #     kps 5-7 (needs an(qc-1), ready after kp 5)
            pairs = [(qc, pr) for qc in range(NQC) for pr in range(2)]
            for pp in pairs:
                st[pp] = {"etiles": [], "pA": None, "pB": None, "rc": None}

            for i, (qc, pr) in enumerate(pairs):
                prev = pairs[i - 1] if i > 0 else None
                is_last = i == len(pairs) - 1
                for kp in range(NKP):
                    ph1_kp(qc, pr, kp)
                    if (qc, pr) == (0, 0):
                        v_proj_tt(2 * kp)
                        v_proj_tt(2 * kp + 1)
                    if not is_last:
                        if prev is not None:
                            if kp < 4:
                                ph2_kts(*prev,
                                        kts=[4 * kp + j for j in range(4)])
                            elif kp == 4:
                                den_mm(*prev)
                            elif kp == 5:
                                norm_mul(*prev)
                    else:
                        # last pair: finish prev early, then self-interleave
                        # attn@V behind its own exps to shorten the tail
                        if kp in (0, 1):
                            ph2_kts(*prev,
                                    kts=[8 * kp + j for j in range(8)])
                        elif kp == 2:
                            den_mm(*prev)
                        elif kp == 3:
                            norm_mul(*prev)
                        else:  # kp 4-7
                            kts = [4 * (kp - 4) + j for j in range(4)]
                            ph2_kts(qc, pr,
                                    kts=[kt for kt in kts if kt < NKT - 2])
                    if pr == 0 and qc >= 1 and kp in (6, 7):
                        outproj_tt(qc - 1, 2 * (kp - 6))
                        outproj_tt(qc - 1, 2 * (kp - 6) + 1)
                if pr == 0 and qc >= 1:
                    reduce_scatter(qc - 1)
                # Q projection for the next chunk after this pair's phase 1
                if pr == 1 and qc < NQC - 1:
                    qk_proj_chunk(qt_t, wq_t, bq_t, xt_q, qc + 1)

            # ---- tail: last pair's remaining attn@V, norm, outproj, RS ----
            last = pairs[-1]
            ph2_kts(*last, kts=[NKT - 2, NKT - 1])
            den_mm(*last)
            norm_mul(*last)
            for tt4 in range(4):
                outproj_tt(NQC - 1, tt4)
            reduce_scatter(NQC - 1)

    nc.compile()
    return nc


def _get_nc():
    global _CACHED_NC
    if _CACHED_NC is None:
        _CACHED_NC = _build_module()
    return _CACHED_NC


def _make_in_maps(query, key, value, Wq, bq, Wk, bk, Wv, bv, Wo, bo):
    query = np.asarray(query, dtype=np.float32)
    key = np.asarray(key, dtype=np.float32)
    value = np.asarray(value, dtype=np.float32)
    Wq = np.asarray(Wq, dtype=np.float32)
    Wk = np.asarray(Wk, dtype=np.float32)
    Wv = np.asarray(Wv, dtype=np.float32)
    Wo = np.asarray(Wo, dtype=np.float32)
    bq = np.asarray(bq, dtype=np.float32)
    bk = np.asarray(bk, dtype=np.float32)
    bv = np.asarray(bv, dtype=np.float32)
    bo = np.asarray(bo, dtype=np.float32)

    # feature-major x^T per batch, fp16, pre-tiled [partition, dtile*token]
    # so each SBUF partition's data is one contiguous DRAM block
    xT = [np.ascontiguousarray(
              t.T.astype(np.float16).reshape(NDT, P, S).transpose(1, 0, 2)
          ).reshape(P, NDT * S)
          for t in (query[0], key[0], value[0], query[1], key[1], value[1])]

    in_maps = []
    for c in range(NCORES):
        b = c // GPB
        g = c % GPB
        sl = slice(g * DS, (g + 1) * DS)
        in_maps.append({
            "xq": xT[3 * b + 0],
            "xk": xT[3 * b + 1],
            "xv": xT[3 * b + 2],
            "wq": Wq[:, sl].astype(np.float16),
            "wk": Wk[:, sl].astype(np.float16),
            "wv": Wv[:, sl].astype(np.float16),
            "wo": Wo[sl, :].astype(np.float16),
            "bq": bq[sl].reshape(DS, 1).copy(),
            "bk": bk[sl].reshape(DS, 1).copy(),
            "bv": bv[sl].reshape(1, DS).copy(),
            "bo": (bo if g == 0 else np.zeros_like(bo)).reshape(1, D).copy(),
        })
    return in_maps


def run(inputs, trace=False, trace_cores=None):
    """Run the SPMD kernel; returns (full_output, BassKernelResults)."""
    nc = _get_nc()
    in_maps = _make_in_maps(**inputs)
    res = run_bass_kernel_spmd(
        nc, in_maps, core_ids=list(range(NCORES)), trace=trace,
        trace_cores=trace_cores)
    out = np.empty((B, S, D), dtype=np.float32)
    for c in range(NCORES):
        b = c // GPB
        g = c % GPB
        o = res.results[c]["out"].astype(np.float32)
        for j in range(4):
            out[b, j * 512 + g * P:j * 512 + (g + 1) * P, :] = \
                o[j * P:(j + 1) * P, :]
    return out, res


def kernel(**inputs):
    out, _ = run(inputs, trace=False)
    return out


# revision 57
# speedup vs baseline: 1.0201x; 1.0201x over previous
"""Multi-head attention (B=2, S=2048, D=1024, H=16) on 8 Trainium2 NeuronCores.

Sharding: batch x head-group. Core c handles batch b = c//4 and heads
[4*(c%4), 4*(c%4)+4) (a 256-wide slice of the QKV projection output and the
matching 256-row slice of Wo). Each core computes its partial output
projection; a 4-way ReduceScatter per batch group sums the partials and
leaves each core with a [512, 1024] row block of the final output, which the
host reassembles.

Per-core dataflow (all matmul operands fp16, fp32 PSUM accumulation):
  - x^T arrives pre-transposed from the host (plain contiguous DMA), split
    across both HWDGE queues (SP: K/V path, ACT: Q path) so the K
    projection starts ~3us in.
  - Q^T, K^T feature-major; K^T zero-padded per head so scores run as
    full-128-contraction plain matmuls. All matmuls stay in plain mode -
    no PE tiling-mode switches anywhere.
  - Scores computed transposed (S^T[k, q] = K_h @ Q_h^T); softmax without
    max-subtraction (exp on ScalarE, 1/sqrt(dh) scale folded in).
  - attn@V via ones-augmented V so each accumulation also produces the
    softmax denominator in a spare PSUM row. A per-pair indicator matmul
    broadcasts the two denominator rows to all 128 partitions; fast DVE
    reciprocal + two muls produce normalized attn^T.
  - The whole kernel is software-pipelined around the ScalarE exp stream
    (the throughput floor): each head-pair's phase-1 score/exp loop has the
    previous pair's attn@V, the denominator matmul, the V/Q projections and
    the previous chunk's output projection injected into its PE slack.
  - Per-q-chunk ReduceScatter overlaps the next chunk's attention.
"""

import numpy as np

import concourse.bass as bass  # noqa: F401  (engine namespaces via nc)
import concourse.mybir as mybir
import concourse.tile as tile
from concourse import bacc
from concourse.bass import _add_dep_helper
from concourse.bass_utils import run_bass_kernel_spmd

F32 = mybir.dt.float32
F16 = mybir.dt.float16
F8 = mybir.dt.float8e4
NP_F8 = mybir.dt.np(F8)
AF = mybir.ActivationFunctionType

B, S, D = 2, 2048, 1024
H, DH = 16, 64
NCORES = 8
GPB = 4                # cores per batch group
HPC = H // GPB         # heads per core
DS = HPC * DH          # 256: per-core slice of the projection output
P = 128
NDT = D // P           # 8 d_model tiles
NTT = S // P           # 16 token tiles
QCH = 512              # q-chunk (PSUM bank = 512 fp32)
NQC = S // QCH         # 4
NKT = S // P           # 16 k tiles
NKP = NKT // 2         # 8 phase-1 iterations per pair
SCALE = float(1.0 / np.sqrt(DH))

REPLICA_GROUPS = [[0, 1, 2, 3], [4, 5, 6, 7]]

_CACHED_NC = None


def _build_module():
    nc = bacc.Bacc("TRN2", target_bir_lowering=False, debug=False,
                   num_devices=NCORES)

    # host-prearranged [partition, dmodel-tile * token] so each load DMA is
    # a straight copy (contiguous 4KB blocks per partition per tile)
    xq_d = nc.dram_tensor("xq", [P, NDT * S], F16, kind="ExternalInput")
    xk_d = nc.dram_tensor("xk", [P, NDT * S], F16, kind="ExternalInput")
    xv_d = nc.dram_tensor("xv", [P, NDT * S], F16, kind="ExternalInput")
    wq_d = nc.dram_tensor("wq", [D, DS], F16, kind="ExternalInput")
    wk_d = nc.dram_tensor("wk", [D, DS], F16, kind="ExternalInput")
    wv_d = nc.dram_tensor("wv", [D, DS], F16, kind="ExternalInput")
    wo_d = nc.dram_tensor("wo", [DS, D], F16, kind="ExternalInput")
    bq_d = nc.dram_tensor("bq", [DS, 1], F32, kind="ExternalInput")
    bk_d = nc.dram_tensor("bk", [DS, 1], F32, kind="ExternalInput")
    bv_d = nc.dram_tensor("bv", [1, DS], F32, kind="ExternalInput")
    bo_d = nc.dram_tensor("bo", [1, D], F32, kind="ExternalInput")

    out_d = nc.dram_tensor("out", [S // GPB, D], F16, kind="ExternalOutput")
    partial_cs = [nc.dram_tensor(f"partial{j}", [4 * P, D], F16)
                  for j in range(4)]
    rs_cs = [nc.dram_tensor(f"rs_out{j}", [P, D], F16)
             for j in range(4)]

    with tile.TileContext(nc) as tc:
        with (
            tc.tile_pool(name="cst", bufs=1) as cst,
            tc.tile_pool(name="xt", bufs=17) as xtp,
            tc.tile_pool(name="exp", bufs=22) as expp,
            tc.tile_pool(name="rcp", bufs=2) as rcpp,
            tc.tile_pool(name="osb", bufs=3) as osbp,
            tc.tile_pool(name="psB", bufs=3, space="PSUM") as psB,
            tc.tile_pool(name="psC", bufs=1, space="PSUM") as psC,
        ):
            # Total PE ordering: chain every matmul to its predecessor
            # (nosync = scheduling-order only). All matmuls are plain mode.
            _real_matmul = nc.tensor.matmul
            _prev_mm = {"inst": None}

            def mm(out, lhsT, rhs, **kw):
                inst = _real_matmul(out, lhsT, rhs, **kw)
                if _prev_mm["inst"] is not None:
                    _add_dep_helper(
                        inst.ins, _prev_mm["inst"].ins,
                        sync=False, reason="pe-order")
                _prev_mm["inst"] = inst
                return inst

            # ---- constants + inputs: SP queue = K/V path, ACT queue = Q --
            wq_t = cst.tile([P, NDT, DS], F16, tag="wq")
            wk_t = cst.tile([P, NDT, DS], F16, tag="wk")
            wv_t = cst.tile([P, NDT, DS], F16, tag="wv")
            wo_t = cst.tile([P, 2, D], F16, tag="wo")
            bq_t = cst.tile([P, 2, 1], F32, tag="bq")
            bk_t = cst.tile([P, 2, 1], F32, tag="bk")
            bv_row = cst.tile([1, DS], F32, tag="bvr")
            bo_row = cst.tile([1, D], F32, tag="bor")

            def load_xt(x_d, eng, tag):
                # per-tile DMAs through a ring pool: tiles become available
                # progressively and xv reuses xk's buffers after K-proj
                tiles = []
                for dt in range(NDT):
                    t = xtp.tile([P, S], F16, tag="xt", name=f"x{tag}{dt}")
                    eng.dma_start(t[:], x_d[:, dt * S:(dt + 1) * S])
                    tiles.append(t)
                return tiles

            # Single queue, strictly in order of first use (HBM bandwidth is
            # shared across queues, so splitting only delays the K/Q path).
            nc.sync.dma_start(wk_t[:], wk_d.rearrange("(a p) n -> p a n", p=P))
            xt_k = load_xt(xk_d, nc.sync, "bigA")
            nc.sync.dma_start(wq_t[:], wq_d.rearrange("(a p) n -> p a n", p=P))
            xt_q = load_xt(xq_d, nc.sync, "bigB")
            nc.sync.dma_start(wv_t[:], wv_d.rearrange("(a p) n -> p a n", p=P))
            nc.sync.dma_start(wo_t[:], wo_d.rearrange("(a p) n -> p a n", p=P))
            nc.sync.dma_start(bo_row[:], bo_d[:])
            xt_v = load_xt(xv_d, nc.sync, "bigA")

            nc.scalar.dma_start(
                bq_t[:], bq_d.rearrange("(a p) o -> p a o", p=P))
            nc.scalar.dma_start(
                bk_t[:], bk_d.rearrange("(a p) o -> p a o", p=P))
            nc.scalar.dma_start(bv_row[:], bv_d[:])

            bv_b = cst.tile([P, 2, 2, DH], F32, tag="bvb")
            bo_b = cst.tile([P, D], F32, tag="bob")
            nc.gpsimd.partition_broadcast(bv_b[:], bv_row[:])
            nc.gpsimd.partition_broadcast(bo_b[:], bo_row[:])

            # ---- resident activations ----
            qt_t = cst.tile([P, 2, S], F16, tag="qt")   # Q^T  (pair, t)
            kz_t = cst.tile([P, HPC, S], F16, tag="kz")  # zero-padded K^T
            nc.vector.memset(kz_t[:], 0.0)
            an_t = cst.tile([P, 2, S], F16, tag="an")   # attn_norm^T

            # Ones-augmented V, token-major (see module docstring).
            vaA_t = cst.tile([P, NTT, 2, P], F16, tag="vaA")
            vaB_t = cst.tile([P, NTT, 2, P], F16, tag="vaB")
            nc.vector.memset(vaA_t[:], 0.0)
            nc.vector.memset(vaB_t[:], 0.0)
            nc.vector.memset(vaA_t[:, :, :, DH:DH + 1], 1.0)
            nc.vector.memset(vaB_t[:, :, :, 0:1], 1.0)

            # Indicator for the denominator select+broadcast matmul.
            ind_t = cst.tile([P, P], F16, tag="ind")
            nc.vector.memset(ind_t[:], 0.0)
            nc.vector.memset(ind_t[DH:DH + 1, 0:DH], 1.0)
            nc.vector.memset(ind_t[0:1, DH:P], 1.0)

            # Denominator staging rows (rows 0/64 per pair; rest must stay
            # zero - the den-matmul contracts all 128 partitions and fp16
            # garbage can be NaN).
            dsb_t = cst.tile([P, 2, QCH], F16, tag="dsb")
            nc.vector.memset(dsb_t[:], 0.0)

            # ---- building blocks ----
            def qk_proj_chunk(dst, w_t, b_t, xt, tci):
                """One 512-token chunk of the feature-major Q/K projection."""
                ts0 = tci * QCH
                ps = psB.tile([P, 2 * QCH], F32, tag="sc")
                for dot in range(2):
                    col = slice(dot * QCH, (dot + 1) * QCH)
                    for dt in range(NDT):
                        mm(
                            ps[:, col],
                            w_t[:, dt, dot * P:(dot + 1) * P],
                            xt[dt][:, ts0:ts0 + QCH],
                            start=(dt == 0), stop=(dt == NDT - 1),
                        )
                if dst is qt_t:
                    for dot in range(2):
                        nc.vector.tensor_scalar_add(
                            dst[:, dot, ts0:ts0 + QCH],
                            ps[:, dot * QCH:(dot + 1) * QCH],
                            b_t[:, dot, :])
                else:  # kz_t: per-head 64-row slices, rest stays zero
                    for h in range(HPC):
                        rows = slice((h % 2) * 64, (h % 2) * 64 + 64)
                        dot = h // 2
                        nc.vector.tensor_scalar_add(
                            kz_t[rows, h, ts0:ts0 + QCH],
                            ps[rows, dot * QCH:(dot + 1) * QCH],
                            b_t[rows, dot, :])

            def v_proj_tt(tt):
                """Ones-augmented V projection for one token tile."""
                ps = psB.tile([P, 2, 2, DH], F32, tag="sc")
                for dt in range(NDT):
                    mm(
                        ps[:],
                        xt_v[dt][:, tt * P:(tt + 1) * P],
                        wv_t[:, dt, :],
                        start=(dt == 0), stop=(dt == NDT - 1),
                    )
                nc.vector.tensor_add(
                    vaA_t[:, tt, :, 0:DH], ps[:, :, 0, :], bv_b[:, :, 0, :])
                nc.vector.tensor_add(
                    vaB_t[:, tt, :, DH:2 * DH], ps[:, :, 1, :],
                    bv_b[:, :, 1, :])

            # per-pair state: etiles, pA, pB
            st = {}

            def ph1_kp(qc, pr, kp):
                """Two score matmul pairs + two exps for (qc, pr), k-pair kp."""
                qs = qc * QCH
                h0, h1 = 2 * pr, 2 * pr + 1
                sc0 = psB.tile([P, 2 * QCH], F32, tag="sc")
                sc1 = psB.tile([P, 2 * QCH], F32, tag="sc")
                for hsel, sc in ((h0, sc0), (h1, sc1)):
                    for j in range(2):
                        ks = (2 * kp + j) * P
                        col = slice(j * QCH, (j + 1) * QCH)
                        mm(sc[:, col], kz_t[:, hsel, ks:ks + P],
                           qt_t[:, pr, qs:qs + QCH], start=True, stop=True)
                e0 = expp.tile([P, 2 * QCH], F16, tag="exp")
                e1 = expp.tile([P, 2 * QCH], F16, tag="exp")
                nc.scalar.activation(e0[:], sc0[:], AF.Exp, scale=SCALE)
                nc.scalar.activation(e1[:], sc1[:], AF.Exp, scale=SCALE)
                st[(qc, pr)]["etiles"].append((e0, e1))

            def ph2_kts(qc, pr, kts):
                """attn@V accumulation matmuls for the given k-tiles."""
                s = st[(qc, pr)]
                if s["pA"] is None:
                    s["pA"] = psC.tile([P, QCH], F32, tag="pA", name="pA")
                    s["pB"] = psC.tile([P, QCH], F32, tag="pB", name="pB")
                pA, pB = s["pA"], s["pB"]
                for kt in kts:
                    e0, e1 = s["etiles"][kt // 2]
                    col = slice((kt % 2) * QCH, (kt % 2 + 1) * QCH)
                    stt = (kt == 0)
                    sp = (kt == NKT - 1)
                    mm(pA[0:DH + 1, :], vaA_t[:, kt, pr, 0:DH + 1],
                       e0[:, col], start=stt, stop=sp, skip_group_check=True)
                    mm(pB[:, :], vaB_t[:, kt, pr, :],
                       e1[:, col], start=stt, stop=sp, skip_group_check=True)
                if kts[-1] == NKT - 1:
                    # stage denominator rows for the den-matmul (fp16 SBUF)
                    nc.vector.tensor_copy(
                        dsb_t[DH:DH + 1, pr, :], pA[DH:DH + 1, :])
                    nc.vector.tensor_copy(
                        dsb_t[0:1, pr, :], pB[0:1, :])

            def den_mm(qc, pr):
                """Denominator broadcast matmul + reciprocal."""
                s = st[(qc, pr)]
                dps = psB.tile([P, 2 * QCH], F32, tag="sc")
                mm(dps[:, 0:QCH], ind_t[:], dsb_t[:, pr, :],
                   start=True, stop=True)
                rc = rcpp.tile([P, QCH], F32, tag="rcp")
                nc.vector.reciprocal_approx_fast(rc[:], dps[:, 0:QCH])
                s["rc"] = rc

            def norm_mul(qc, pr):
                s = st[(qc, pr)]
                qs = qc * QCH
                rc, pA, pB = s["rc"], s["pA"], s["pB"]
                nc.vector.tensor_mul(
                    an_t[0:DH, pr, qs:qs + QCH], pA[0:DH, :], rc[0:DH, :])
                nc.vector.tensor_mul(
                    an_t[DH:P, pr, qs:qs + QCH], pB[DH:P, :], rc[DH:P, :])

            def outproj_tt(qc, tt4):
                tt = qc * (QCH // P) + tt4
                po = psB.tile([P, 2 * QCH], F32, tag="sc")
                for half in range(2):
                    for pr in range(2):
                        mm(po[:, half * QCH:(half + 1) * QCH],
                           an_t[:, pr, tt * P:(tt + 1) * P],
                           wo_t[:, pr, half * QCH:(half + 1) * QCH],
                           start=(pr == 0), stop=(pr == 1))
                ob = osbp.tile([P, D], F16, tag="osb")
                nc.vector.tensor_add(ob[:], po[:], bo_b[:])
                nc.sync.dma_start(
                    partial_cs[qc][tt4 * P:(tt4 + 1) * P, :], ob[:])

            def reduce_scatter(qc):
                nc.gpsimd.collective_compute(
                    "ReduceScatter",
                    mybir.AluOpType.add,
                    replica_groups=REPLICA_GROUPS,
                    ins=[partial_cs[qc][:]],
                    outs=[rs_cs[qc][:]],
                )
                nc.sync.dma_start(out_d[qc * P:(qc + 1) * P, :],
                                  rs_cs[qc][:])

            # ---- head: K projection, then Q chunk 0 ----
            for tci in range(NQC):
                qk_proj_chunk(kz_t, wk_t, bk_t, xt_k, tci)
            qk_proj_chunk(qt_t, wq_t, bq_t, xt_q, 0)

            # ---- software-pipelined attention ----
            # pair p's phase 1 (exp-paced) carries, per kp:
            #   - previous pair's attn@V (front-loaded, 4 k-tiles on kp 0-3)
            #   - previous pair's den-matmul at kp 4, norm muls at kp 5
            #   - pair (0,0): V projection, 2 token tiles per kp
            #   - pairs (qc,1): Q projection chunk qc+1, spread over kps
            #   - pairs (qc,0), qc>=1: output projection of chunk qc-1 on
            #     kps 5-7 (needs an(qc-1), ready after kp 5)
            pairs = [(qc, pr) for qc in range(NQC) for pr in range(2)]
            for pp in pairs:
                st[pp] = {"etiles": [], "pA": None, "pB": None, "rc": None}

            for i, (qc, pr) in enumerate(pairs):
                prev = pairs[i - 1] if i > 0 else None
                is_last = i == len(pairs) - 1
                for kp in range(NKP):
                    ph1_kp(qc, pr, kp)
                    if (qc, pr) == (0, 0):
                        v_proj_tt(2 * kp)
                        v_proj_tt(2 * kp + 1)
                    if not is_last:
                        if prev is not None:
                            if kp < 4:
                                ph2_kts(*prev,
                                        kts=[4 * kp + j for j in range(4)])
                            elif kp == 4:
                                den_mm(*prev)
                            elif kp == 5:
                                norm_mul(*prev)
                    else:
                        # last pair: finish prev early, then self-interleave
                        # attn@V behind its own exps to shorten the tail
                        if kp in (0, 1):
                            ph2_kts(*prev,
                                    kts=[8 * kp + j for j in range(8)])
                        elif kp == 2:
                            den_mm(*prev)
                        elif kp == 3:
                            norm_mul(*prev)
                        else:  # kp 4-7
                            kts = [4 * (kp - 4) + j for j in range(4)]
                            ph2_kts(qc, pr,
                                    kts=[kt for kt in kts if kt < NKT - 2])
                    if pr == 0 and qc >= 1 and kp in (6, 7):
                        outproj_tt(qc - 1, 2 * (kp - 6))
                        outproj_tt(qc - 1, 2 * (kp - 6) + 1)
                if pr == 0 and qc >= 1:
                    reduce_scatter(qc - 1)
                # Q projection for the next chunk after this pair's phase 1
                if pr == 1 and qc < NQC - 1:
                    qk_proj_chunk(qt_t, wq_t, bq_t, xt_q, qc + 1)

            # ---- tail: last pair's remaining attn@V, norm, outproj, RS ----
            last = pairs[-1]
            ph2_kts(*last, kts=[NKT - 2, NKT - 1])
            den_mm(*last)
            norm_mul(*last)
            for tt4 in range(4):
                outproj_tt(NQC - 1, tt4)
            reduce_scatter(NQC - 1)

    nc.compile()
    return nc


def _get_nc():
    global _CACHED_NC
    if _CACHED_NC is None:
        _CACHED_NC = _build_module()
    return _CACHED_NC


def _make_in_maps(query, key, value, Wq, bq, Wk, bk, Wv, bv, Wo, bo):
    query = np.asarray(query, dtype=np.float32)
    key = np.asarray(key, dtype=np.float32)
    value = np.asarray(value, dtype=np.float32)
    Wq = np.asarray(Wq, dtype=np.float32)
    Wk = np.asarray(Wk, dtype=np.float32)
    Wv = np.asarray(Wv, dtype=np.float32)
    Wo = np.asarray(Wo, dtype=np.float32)
    bq = np.asarray(bq, dtype=np.float32)
    bk = np.asarray(bk, dtype=np.float32)
    bv = np.asarray(bv, dtype=np.float32)
    bo = np.asarray(bo, dtype=np.float32)

    # feature-major x^T per batch, fp16, pre-tiled [partition, dtile*token]
    # so each SBUF partition's data is one contiguous DRAM block
    xT = [np.ascontiguousarray(
              t.T.astype(np.float16).reshape(NDT, P, S).transpose(1, 0, 2)
          ).reshape(P, NDT * S)
          for t in (query[0], key[0], value[0], query[1], key[1], value[1])]

    in_maps = []
    for c in range(NCORES):
        b = c // GPB
        g = c % GPB
        sl = slice(g * DS, (g + 1) * DS)
        in_maps.append({
            "xq": xT[3 * b + 0],
            "xk": xT[3 * b + 1],
            "xv": xT[3 * b + 2],
            "wq": Wq[:, sl].astype(np.float16),
            "wk": Wk[:, sl].astype(np.float16),
            "wv": Wv[:, sl].astype(np.float16),
            "wo": Wo[sl, :].astype(np.float16),
            "bq": bq[sl].reshape(DS, 1).copy(),
            "bk": bk[sl].reshape(DS, 1).copy(),
            "bv": bv[sl].reshape(1, DS).copy(),
            "bo": (bo if g == 0 else np.zeros_like(bo)).reshape(1, D).copy(),
        })
    return in_maps


def run(inputs, trace=False, trace_cores=None):
    """Run the SPMD kernel; returns (full_output, BassKernelResults)."""
    nc = _get_nc()
    in_maps = _make_in_maps(**inputs)
    res = run_bass_kernel_spmd(
        nc, in_maps, core_ids=list(range(NCORES)), trace=trace,
        trace_cores=trace_cores)
    out = np.empty((B, S, D), dtype=np.float32)
    for c in range(NCORES):
        b = c // GPB
        g = c % GPB
        o = res.results[c]["out"].astype(np.float32)
        for j in range(4):
            out[b, j * 512 + g * P:j * 512 + (g + 1) * P, :] = \
                o[j * P:(j + 1) * P, :]
    return out, res


def kernel(**inputs):
    out, _ = run(inputs, trace=False)
    return out


# revision 59
# speedup vs baseline: 1.0464x; 1.0258x over previous
"""Multi-head attention (B=2, S=2048, D=1024, H=16) on 8 Trainium2 NeuronCores.

Sharding: batch x head-group. Core c handles batch b = c//4 and heads
[4*(c%4), 4*(c%4)+4) (a 256-wide slice of the QKV projection output and the
matching 256-row slice of Wo). Each core computes its partial output
projection; a 4-way ReduceScatter per batch group sums the partials and
leaves each core with a [512, 1024] row block of the final output, which the
host reassembles.

Per-core dataflow (all matmul operands fp16, fp32 PSUM accumulation):
  - x^T arrives pre-transposed from the host (plain contiguous DMA), split
    across both HWDGE queues (SP: K/V path, ACT: Q path) so the K
    projection starts ~3us in.
  - Q^T, K^T feature-major; K^T zero-padded per head so scores run as
    full-128-contraction plain matmuls. All matmuls stay in plain mode -
    no PE tiling-mode switches anywhere.
  - Scores computed transposed (S^T[k, q] = K_h @ Q_h^T); softmax without
    max-subtraction (exp on ScalarE, 1/sqrt(dh) scale folded in).
  - attn@V via ones-augmented V so each accumulation also produces the
    softmax denominator in a spare PSUM row. A per-pair indicator matmul
    broadcasts the two denominator rows to all 128 partitions; fast DVE
    reciprocal + two muls produce normalized attn^T.
  - The whole kernel is software-pipelined around the ScalarE exp stream
    (the throughput floor): each head-pair's phase-1 score/exp loop has the
    previous pair's attn@V, the denominator matmul, the V/Q projections and
    the previous chunk's output projection injected into its PE slack.
  - Per-q-chunk ReduceScatter overlaps the next chunk's attention.
"""

import numpy as np

import concourse.bass as bass  # noqa: F401  (engine namespaces via nc)
import concourse.mybir as mybir
import concourse.tile as tile
from concourse import bacc
from concourse.bass import _add_dep_helper
from concourse.bass_utils import run_bass_kernel_spmd

F32 = mybir.dt.float32
F16 = mybir.dt.float16
F8 = mybir.dt.float8e4
NP_F8 = mybir.dt.np(F8)
AF = mybir.ActivationFunctionType

B, S, D = 2, 2048, 1024
H, DH = 16, 64
NCORES = 8
GPB = 4                # cores per batch group
HPC = H // GPB         # heads per core
DS = HPC * DH          # 256: per-core slice of the projection output
P = 128
NDT = D // P           # 8 d_model tiles
NTT = S // P           # 16 token tiles
QCH = 512              # q-chunk (PSUM bank = 512 fp32)
NQC = S // QCH         # 4
NKT = S // P           # 16 k tiles
NKP = NKT // 2         # 8 phase-1 iterations per pair
SCALE = float(1.0 / np.sqrt(DH))

REPLICA_GROUPS = [[0, 1, 2, 3], [4, 5, 6, 7]]

_CACHED_NC = None


def _build_module():
    nc = bacc.Bacc("TRN2", target_bir_lowering=False, debug=False,
                   num_devices=NCORES)

    # host-prearranged [partition, dmodel-tile * token] so each load DMA is
    # a straight copy (contiguous 4KB blocks per partition per tile)
    xq_d = nc.dram_tensor("xq", [P, NDT * S], F16, kind="ExternalInput")
    xk_d = nc.dram_tensor("xk", [P, NDT * S], F16, kind="ExternalInput")
    xv_d = nc.dram_tensor("xv", [P, NDT * S], F16, kind="ExternalInput")
    wq_d = nc.dram_tensor("wq", [D, DS], F16, kind="ExternalInput")
    wk_d = nc.dram_tensor("wk", [D, DS], F16, kind="ExternalInput")
    wv_d = nc.dram_tensor("wv", [D, DS], F16, kind="ExternalInput")
    wo_d = nc.dram_tensor("wo", [DS, D], F16, kind="ExternalInput")
    bq_d = nc.dram_tensor("bq", [DS, 1], F32, kind="ExternalInput")
    bk_d = nc.dram_tensor("bk", [DS, 1], F32, kind="ExternalInput")
    bv_d = nc.dram_tensor("bv", [1, DS], F32, kind="ExternalInput")
    bo_d = nc.dram_tensor("bo", [1, D], F32, kind="ExternalInput")

    out_d = nc.dram_tensor("out", [S // GPB, D], F16, kind="ExternalOutput")
    partial_cs = [nc.dram_tensor(f"partial{j}", [4 * P, D], F16)
                  for j in range(4)]
    rs_cs = [nc.dram_tensor(f"rs_out{j}", [P, D], F16)
             for j in range(4)]

    with tile.TileContext(nc) as tc:
        with (
            tc.tile_pool(name="cst", bufs=1) as cst,
            tc.tile_pool(name="xt", bufs=17) as xtp,
            tc.tile_pool(name="exp", bufs=22) as expp,
            tc.tile_pool(name="rcp", bufs=2) as rcpp,
            tc.tile_pool(name="osb", bufs=3) as osbp,
            tc.tile_pool(name="psB", bufs=3, space="PSUM") as psB,
            tc.tile_pool(name="psC", bufs=1, space="PSUM") as psC,
        ):
            # Total PE ordering: chain every matmul to its predecessor
            # (nosync = scheduling-order only). All matmuls are plain mode.
            _real_matmul = nc.tensor.matmul
            _prev_mm = {"inst": None}

            def mm(out, lhsT, rhs, **kw):
                inst = _real_matmul(out, lhsT, rhs, **kw)
                if _prev_mm["inst"] is not None:
                    _add_dep_helper(
                        inst.ins, _prev_mm["inst"].ins,
                        sync=False, reason="pe-order")
                _prev_mm["inst"] = inst
                return inst

            # ---- constants + inputs: SP queue = K/V path, ACT queue = Q --
            wq_t = cst.tile([P, NDT, DS], F16, tag="wq")
            wk_t = cst.tile([P, NDT, DS], F16, tag="wk")
            wv_t = cst.tile([P, NDT, DS], F16, tag="wv")
            wo_t = cst.tile([P, 2, D], F16, tag="wo")
            bq_t = cst.tile([P, 2, 1], F32, tag="bq")
            bk_t = cst.tile([P, 2, 1], F32, tag="bk")
            bv_row = cst.tile([1, DS], F32, tag="bvr")
            bo_row = cst.tile([1, D], F32, tag="bor")

            def load_xt(x_d, eng, tag):
                # per-tile DMAs through a ring pool: tiles become available
                # progressively and xv reuses xk's buffers after K-proj
                tiles = []
                for dt in range(NDT):
                    t = xtp.tile([P, S], F16, tag="xt", name=f"x{tag}{dt}")
                    eng.dma_start(t[:], x_d[:, dt * S:(dt + 1) * S])
                    tiles.append(t)
                return tiles

            # Single queue, strictly in order of first use (HBM bandwidth is
            # shared across queues, so splitting only delays the K/Q path).
            nc.sync.dma_start(wk_t[:], wk_d.rearrange("(a p) n -> p a n", p=P))
            xt_k = load_xt(xk_d, nc.sync, "bigA")
            nc.sync.dma_start(wq_t[:], wq_d.rearrange("(a p) n -> p a n", p=P))
            xt_q = load_xt(xq_d, nc.sync, "bigB")
            nc.sync.dma_start(wv_t[:], wv_d.rearrange("(a p) n -> p a n", p=P))
            nc.sync.dma_start(wo_t[:], wo_d.rearrange("(a p) n -> p a n", p=P))
            nc.sync.dma_start(bo_row[:], bo_d[:])
            xt_v = load_xt(xv_d, nc.sync, "bigA")

            nc.scalar.dma_start(
                bq_t[:], bq_d.rearrange("(a p) o -> p a o", p=P))
            nc.scalar.dma_start(
                bk_t[:], bk_d.rearrange("(a p) o -> p a o", p=P))
            nc.scalar.dma_start(bv_row[:], bv_d[:])

            bv_b = cst.tile([P, 2, 2, DH], F32, tag="bvb")
            bo_b = cst.tile([P, D], F32, tag="bob")
            nc.gpsimd.partition_broadcast(bv_b[:], bv_row[:])
            nc.gpsimd.partition_broadcast(bo_b[:], bo_row[:])

            # ---- resident activations ----
            qt_t = cst.tile([P, 2, S], F16, tag="qt")   # Q^T  (pair, t)
            kz_t = cst.tile([P, HPC, S], F16, tag="kz")  # zero-padded K^T
            nc.vector.memset(kz_t[:], 0.0)
            an_t = cst.tile([P, 2, S], F16, tag="an")   # attn_norm^T

            # Ones-augmented V, token-major (see module docstring).
            vaA_t = cst.tile([P, NTT, 2, P], F16, tag="vaA")
            vaB_t = cst.tile([P, NTT, 2, P], F16, tag="vaB")
            nc.vector.memset(vaA_t[:], 0.0)
            nc.vector.memset(vaB_t[:], 0.0)
            nc.vector.memset(vaA_t[:, :, :, DH:DH + 1], 1.0)
            nc.vector.memset(vaB_t[:, :, :, 0:1], 1.0)

            # Indicator for the denominator select+broadcast matmul.
            ind_t = cst.tile([P, P], F16, tag="ind")
            nc.vector.memset(ind_t[:], 0.0)
            nc.vector.memset(ind_t[DH:DH + 1, 0:DH], 1.0)
            nc.vector.memset(ind_t[0:1, DH:P], 1.0)

            # Denominator staging rows (rows 0/64 per pair; rest must stay
            # zero - the den-matmul contracts all 128 partitions and fp16
            # garbage can be NaN).
            dsb_t = cst.tile([P, 2, QCH], F16, tag="dsb")
            nc.vector.memset(dsb_t[:], 0.0)

            # ---- building blocks ----
            def qk_proj_dot(dst, w_t, b_t, xt, tci, dot, ps):
                """Half (one 128-feature dot) of a Q/K projection chunk."""
                ts0 = tci * QCH
                col = slice(dot * QCH, (dot + 1) * QCH)
                for dt in range(NDT):
                    mm(
                        ps[:, col],
                        w_t[:, dt, dot * P:(dot + 1) * P],
                        xt[dt][:, ts0:ts0 + QCH],
                        start=(dt == 0), stop=(dt == NDT - 1),
                    )
                if dst is qt_t:
                    nc.vector.tensor_scalar_add(
                        dst[:, dot, ts0:ts0 + QCH],
                        ps[:, dot * QCH:(dot + 1) * QCH],
                        b_t[:, dot, :])
                else:  # kz_t: per-head 64-row slices, rest stays zero
                    for h in (2 * dot, 2 * dot + 1):
                        rows = slice((h % 2) * 64, (h % 2) * 64 + 64)
                        nc.vector.tensor_scalar_add(
                            kz_t[rows, h, ts0:ts0 + QCH],
                            ps[rows, dot * QCH:(dot + 1) * QCH],
                            b_t[rows, dot, :])

            def qk_proj_chunk(dst, w_t, b_t, xt, tci):
                ps = psB.tile([P, 2 * QCH], F32, tag="sc", name="qkps")
                for dot in range(2):
                    qk_proj_dot(dst, w_t, b_t, xt, tci, dot, ps)

            def v_proj_tt(tt):
                """Ones-augmented V projection for one token tile."""
                ps = psB.tile([P, 2, 2, DH], F32, tag="sc")
                for dt in range(NDT):
                    mm(
                        ps[:],
                        xt_v[dt][:, tt * P:(tt + 1) * P],
                        wv_t[:, dt, :],
                        start=(dt == 0), stop=(dt == NDT - 1),
                    )
                nc.vector.tensor_add(
                    vaA_t[:, tt, :, 0:DH], ps[:, :, 0, :], bv_b[:, :, 0, :])
                nc.vector.tensor_add(
                    vaB_t[:, tt, :, DH:2 * DH], ps[:, :, 1, :],
                    bv_b[:, :, 1, :])

            # per-pair state: etiles, pA, pB
            st = {}

            def ph1_kp(qc, pr, kp):
                """Two score matmul pairs + two exps for (qc, pr), k-pair kp."""
                qs = qc * QCH
                h0, h1 = 2 * pr, 2 * pr + 1
                sc0 = psB.tile([P, 2 * QCH], F32, tag="sc")
                sc1 = psB.tile([P, 2 * QCH], F32, tag="sc")
                for hsel, sc in ((h0, sc0), (h1, sc1)):
                    for j in range(2):
                        ks = (2 * kp + j) * P
                        col = slice(j * QCH, (j + 1) * QCH)
                        mm(sc[:, col], kz_t[:, hsel, ks:ks + P],
                           qt_t[:, pr, qs:qs + QCH], start=True, stop=True)
                e0 = expp.tile([P, 2 * QCH], F16, tag="exp")
                e1 = expp.tile([P, 2 * QCH], F16, tag="exp")
                nc.scalar.activation(e0[:], sc0[:], AF.Exp, scale=SCALE)
                nc.scalar.activation(e1[:], sc1[:], AF.Exp, scale=SCALE)
                st[(qc, pr)]["etiles"].append((e0, e1))

            def ph2_kts(qc, pr, kts):
                """attn@V accumulation matmuls for the given k-tiles."""
                s = st[(qc, pr)]
                if s["pA"] is None:
                    s["pA"] = psC.tile([P, QCH], F32, tag="pA", name="pA")
                    s["pB"] = psC.tile([P, QCH], F32, tag="pB", name="pB")
                pA, pB = s["pA"], s["pB"]
                for kt in kts:
                    e0, e1 = s["etiles"][kt // 2]
                    col = slice((kt % 2) * QCH, (kt % 2 + 1) * QCH)
                    stt = (kt == 0)
                    sp = (kt == NKT - 1)
                    mm(pA[0:DH + 1, :], vaA_t[:, kt, pr, 0:DH + 1],
                       e0[:, col], start=stt, stop=sp, skip_group_check=True)
                    mm(pB[:, :], vaB_t[:, kt, pr, :],
                       e1[:, col], start=stt, stop=sp, skip_group_check=True)
                if kts[-1] == NKT - 1:
                    # stage denominator rows for the den-matmul (fp16 SBUF)
                    nc.vector.tensor_copy(
                        dsb_t[DH:DH + 1, pr, :], pA[DH:DH + 1, :])
                    nc.vector.tensor_copy(
                        dsb_t[0:1, pr, :], pB[0:1, :])

            def den_mm(qc, pr):
                """Denominator broadcast matmul + reciprocal."""
                s = st[(qc, pr)]
                dps = psB.tile([P, 2 * QCH], F32, tag="sc")
                mm(dps[:, 0:QCH], ind_t[:], dsb_t[:, pr, :],
                   start=True, stop=True)
                rc = rcpp.tile([P, QCH], F32, tag="rcp")
                nc.vector.reciprocal_approx_fast(rc[:], dps[:, 0:QCH])
                s["rc"] = rc

            def norm_mul(qc, pr):
                s = st[(qc, pr)]
                qs = qc * QCH
                rc, pA, pB = s["rc"], s["pA"], s["pB"]
                nc.vector.tensor_mul(
                    an_t[0:DH, pr, qs:qs + QCH], pA[0:DH, :], rc[0:DH, :])
                nc.vector.tensor_mul(
                    an_t[DH:P, pr, qs:qs + QCH], pB[DH:P, :], rc[DH:P, :])

            def outproj_tt(qc, tt4):
                tt = qc * (QCH // P) + tt4
                po = psB.tile([P, 2 * QCH], F32, tag="sc")
                for half in range(2):
                    for pr in range(2):
                        mm(po[:, half * QCH:(half + 1) * QCH],
                           an_t[:, pr, tt * P:(tt + 1) * P],
                           wo_t[:, pr, half * QCH:(half + 1) * QCH],
                           start=(pr == 0), stop=(pr == 1))
                ob = osbp.tile([P, D], F16, tag="osb")
                nc.vector.tensor_add(ob[:], po[:], bo_b[:])
                nc.sync.dma_start(
                    partial_cs[qc][tt4 * P:(tt4 + 1) * P, :], ob[:])

            def reduce_scatter(qc):
                nc.gpsimd.collective_compute(
                    "ReduceScatter",
                    mybir.AluOpType.add,
                    replica_groups=REPLICA_GROUPS,
                    ins=[partial_cs[qc][:]],
                    outs=[rs_cs[qc][:]],
                )
                nc.sync.dma_start(out_d[qc * P:(qc + 1) * P, :],
                                  rs_cs[qc][:])

            # ---- head: K projection, then Q chunk 0 ----
            for tci in range(NQC):
                qk_proj_chunk(kz_t, wk_t, bk_t, xt_k, tci)
            qk_proj_chunk(qt_t, wq_t, bq_t, xt_q, 0)

            # ---- software-pipelined attention ----
            # pair p's phase 1 (exp-paced) carries, per kp:
            #   - previous pair's attn@V (front-loaded, 4 k-tiles on kp 0-3)
            #   - previous pair's den-matmul at kp 4, norm muls at kp 5
            #   - pair (0,0): V projection, 2 token tiles per kp
            #   - pairs (qc,1): Q projection chunk qc+1, spread over kps
            #   - pairs (qc,0), qc>=1: output projection of chunk qc-1 on
            #     kps 5-7 (needs an(qc-1), ready after kp 5)
            pairs = [(qc, pr) for qc in range(NQC) for pr in range(2)]
            for pp in pairs:
                st[pp] = {"etiles": [], "pA": None, "pB": None, "rc": None}

            for i, (qc, pr) in enumerate(pairs):
                prev = pairs[i - 1] if i > 0 else None
                is_last = i == len(pairs) - 1
                for kp in range(NKP):
                    ph1_kp(qc, pr, kp)
                    if (qc, pr) == (0, 0):
                        v_proj_tt(2 * kp)
                        v_proj_tt(2 * kp + 1)
                    if not is_last:
                        if prev is not None:
                            if kp < 4:
                                ph2_kts(*prev,
                                        kts=[4 * kp + j for j in range(4)])
                            elif kp == 4:
                                den_mm(*prev)
                            elif kp == 5:
                                norm_mul(*prev)
                    else:
                        # last pair: finish prev early, then self-interleave
                        # attn@V behind its own exps to shorten the tail
                        if kp in (0, 1):
                            ph2_kts(*prev,
                                    kts=[8 * kp + j for j in range(8)])
                        elif kp == 2:
                            den_mm(*prev)
                        elif kp == 3:
                            norm_mul(*prev)
                        else:  # kp 4-7
                            kts = [4 * (kp - 4) + j for j in range(4)]
                            ph2_kts(qc, pr,
                                    kts=[kt for kt in kts if kt < NKT - 2])
                    if pr == 0 and qc >= 1 and kp in (6, 7):
                        outproj_tt(qc - 1, 2 * (kp - 6))
                        outproj_tt(qc - 1, 2 * (kp - 6) + 1)
                    # next chunk's Q projection rides the light kps 6-7
                    if pr == 1 and qc < NQC - 1 and kp in (6, 7):
                        if kp == 6:
                            st[(qc, pr)]["qps"] = psB.tile(
                                [P, 2 * QCH], F32, tag="sc", name="qkps")
                        qk_proj_dot(qt_t, wq_t, bq_t, xt_q, qc + 1,
                                    kp - 6, st[(qc, pr)]["qps"])
                if pr == 0 and qc >= 1:
                    reduce_scatter(qc - 1)

            # ---- tail: last pair's remaining attn@V, norm, outproj, RS ----
            last = pairs[-1]
            ph2_kts(*last, kts=[NKT - 2, NKT - 1])
            den_mm(*last)
            norm_mul(*last)
            for tt4 in range(4):
                outproj_tt(NQC - 1, tt4)
            reduce_scatter(NQC - 1)

    nc.compile()
    return nc


def _get_nc():
    global _CACHED_NC
    if _CACHED_NC is None:
        _CACHED_NC = _build_module()
    return _CACHED_NC


def _make_in_maps(query, key, value, Wq, bq, Wk, bk, Wv, bv, Wo, bo):
    query = np.asarray(query, dtype=np.float32)
    key = np.asarray(key, dtype=np.float32)
    value = np.asarray(value, dtype=np.float32)
    Wq = np.asarray(Wq, dtype=np.float32)
    Wk = np.asarray(Wk, dtype=np.float32)
    Wv = np.asarray(Wv, dtype=np.float32)
    Wo = np.asarray(Wo, dtype=np.float32)
    bq = np.asarray(bq, dtype=np.float32)
    bk = np.asarray(bk, dtype=np.float32)
    bv = np.asarray(bv, dtype=np.float32)
    bo = np.asarray(bo, dtype=np.float32)

    # feature-major x^T per batch, fp16, pre-tiled [partition, dtile*token]
    # so each SBUF partition's data is one contiguous DRAM block
    xT = [np.ascontiguousarray(
              t.T.astype(np.float16).reshape(NDT, P, S).transpose(1, 0, 2)
          ).reshape(P, NDT * S)
          for t in (query[0], key[0], value[0], query[1], key[1], value[1])]

    in_maps = []
    for c in range(NCORES):
        b = c // GPB
        g = c % GPB
        sl = slice(g * DS, (g + 1) * DS)
        in_maps.append({
            "xq": xT[3 * b + 0],
            "xk": xT[3 * b + 1],
            "xv": xT[3 * b + 2],
            "wq": Wq[:, sl].astype(np.float16),
            "wk": Wk[:, sl].astype(np.float16),
            "wv": Wv[:, sl].astype(np.float16),
            "wo": Wo[sl, :].astype(np.float16),
            "bq": bq[sl].reshape(DS, 1).copy(),
            "bk": bk[sl].reshape(DS, 1).copy(),
            "bv": bv[sl].reshape(1, DS).copy(),
            "bo": (bo if g == 0 else np.zeros_like(bo)).reshape(1, D).copy(),
        })
    return in_maps


def run(inputs, trace=False, trace_cores=None):
    """Run the SPMD kernel; returns (full_output, BassKernelResults)."""
    nc = _get_nc()
    in_maps = _make_in_maps(**inputs)
    res = run_bass_kernel_spmd(
        nc, in_maps, core_ids=list(range(NCORES)), trace=trace,
        trace_cores=trace_cores)
    out = np.empty((B, S, D), dtype=np.float32)
    for c in range(NCORES):
        b = c // GPB
        g = c % GPB
        o = res.results[c]["out"].astype(np.float32)
        for j in range(4):
            out[b, j * 512 + g * P:j * 512 + (g + 1) * P, :] = \
                o[j * P:(j + 1) * P, :]
    return out, res


def kernel(**inputs):
    out, _ = run(inputs, trace=False)
    return out
